# revision 1
# baseline (speedup 1.0000x reference)
"""Trainium2 Bass kernel for nn_DockingTimeModel (2-layer GINE GNN + mean-pool
+ MLP head), single merged SPMD launch on 8 NeuronCores.

Sharding: data-parallel over graphs. Core c owns graphs [512c, 512(c+1)) and
their (contiguous, `batch` is sorted) node range; edges live on the core owning
their dst node. Per layer: dma_gather x[src] rows from host-compacted int16
tables -> edge linear on PE (bias folded via ones-row) -> relu(gather+lin) ->
dma_scatter_add into an HBM accumulator by local dst (dst-unique per chunk;
Tile serializes chunks so HBM read-modify-write never races) -> feat-major node
MLP on PE. Between layers, each core pre-gathers the h1 rows every peer needs
and exchanges them with one AllToAll; layer 2 gathers from the received
compact table. Mean-pool via per-tile indicator matmuls -> partials ->
one dma_gather + reduce; MLP head on-chip; output [1, 512] per core.
"""
import sys

sys.path.insert(0, "/opt/trn_rl_repo")

import math
from contextlib import ExitStack
from dataclasses import dataclass

import numpy as np

from concourse import bacc, bass, mybir, tile
from concourse import bass_utils
from concourse.masks import make_identity

F32 = mybir.dt.float32
I16 = mybir.dt.int16
AF = mybir.ActivationFunctionType
ALU = mybir.AluOpType

C = 8
P = 128
ND = 64
ED = 16
EMB = 128
USR = 12


def _wrap16(idx):
    L = len(idx)
    assert L % 16 == 0
    a = np.asarray(idx, np.int16).reshape(L // 16, 16).T
    return np.tile(a, (8, 1))


@dataclass
class Stream:
    """One layer's edge stream layout: groups of (n_chunks, chunk_size,
    table_id, kind) with kind in {"s","p"} (single / paired dst rows)."""
    groups: list          # [(k, CH, table_id, kind)]
    ESH: int = 0
    DSH: int = 0          # total didx entries

    @property
    def chunks(self):
        out = []
        off = doff = 0
        for k, CH, tb, kind in self.groups:
            nd = CH if kind == "s" else CH // 2
            for i in range(k):
                out.append((off, CH, tb, kind, doff))
                off += CH
                doff += nd
        return out


@dataclass
class CFG:
    TAB0: int
    TAB1: int
    s1: Stream
    s2: Stream
    N_SH: int
    B: int                # A2A block rows per (dst,src) pair
    NCH: int = 512
    GS: int = 512
    GSP: int = 512
    GW: int = 8
    PG: int = 2
    n_pool_idx: int = 0

    @property
    def NT(self):
        return self.N_SH // P


def _split_group(pos, dloc, split, gi):
    """Edges of table-group gi -> (paired a, paired b, singles) edge-index
    arrays. a[i], b[i] go to nodes (2m, 2m+1) for some m."""
    sel = np.nonzero((pos < split) if gi == 0 else (pos >= split))[0]
    if not len(sel):
        return (np.zeros(0, np.int64),) * 3
    d = dloc[sel]
    order = sel[np.argsort(d, kind="stable")]
    sd = dloc[order]
    pid = sd // 2
    # within each node: contiguous run; within each pair id: a-run then b-run
    a_list, b_list, s_list = [], [], []
    bounds = np.nonzero(np.diff(pid))[0] + 1
    startp = np.concatenate([[0], bounds])
    endp = np.concatenate([bounds, [len(sd)]])
    for s0, e0 in zip(startp, endp):
        seg = order[s0:e0]
        segd = sd[s0:e0]
        even = seg[segd % 2 == 0]
        odd = seg[segd % 2 == 1]
        npair = min(len(even), len(odd))
        a_list.append(even[:npair])
        b_list.append(odd[:npair])
        s_list.append(even[npair:])
        s_list.append(odd[npair:])
    cat = lambda L: (np.concatenate(L) if L else np.zeros(0, np.int64))
    return cat(a_list), cat(b_list), cat(s_list)


def _mk_stream(pos_all, dloc_all, ea_all, split, CHUNK_MAX):
    """Build per-layer stream shape: for each table group, a paired subgroup
    (dst = node-pair rows, 512B descs) and a singles subgroup."""
    Cn = len(pos_all)
    groups = []
    for gi in range(2):
        pr_n, pr_m, sg_n, sg_m = [], [1], [], [1]
        for c in range(Cn):
            a, b, sng = _split_group(pos_all[c], dloc_all[c], split, gi)
            pr_n.append(len(a))
            sg_n.append(len(sng))
            if len(a):
                pr_m.append(int(np.bincount(dloc_all[c][a] // 2).max()))
            if len(sng):
                sg_m.append(int(np.bincount(dloc_all[c][sng]).max()))
        # paired subgroup: CH counts EDGES (2 per pair); chunk mult of 256
        if max(pr_n) > 0:
            npmax = max(pr_n)  # pairs
            k = max(int(math.ceil(2 * npmax / (CHUNK_MAX * 0.95))), max(pr_m))
            CH = int(math.ceil(npmax / k * 1.08 / 128) * 256)
            CH = max(CH, 512)
            groups.append((k, CH, gi, "p"))
        if max(sg_n) > 0:
            nsmax = max(sg_n)
            k = max(int(math.ceil(nsmax / (CHUNK_MAX * 0.95))), max(sg_m))
            CH = int(math.ceil(nsmax / k * 1.15 / 128) * 128)
            CH = max(CH, 256)
            groups.append((k, CH, gi, "s"))
    st = Stream(groups=groups)
    st.ESH = sum(k * ch for k, ch, _, _ in st.groups)
    st.DSH = sum(k * (ch if kind == "s" else ch // 2)
                 for k, ch, _, kind in st.groups)
    st.DSH = int(math.ceil(st.DSH / 16) * 16)
    return st


def _assign_chunks(ids, k, CAP, retry=6):
    """Chunk assignment: for items with target ids (dst nodes or pair ids),
    id-unique per chunk via (rank_within_id + hash(id)) % k."""
    order = np.argsort(ids, kind="stable")
    si = ids[order]
    rank = np.arange(len(si)) - np.searchsorted(si, si, side="left")
    for salt in range(retry):
        cid = (rank + (si * (2654435761 + salt * 97)) % k) % k
        if np.bincount(cid, minlength=k).max() <= CAP:
            return order, cid
    raise AssertionError(
        f"chunk overflow {np.bincount(cid, minlength=k).max()} > {CAP}")


def _fill_stream(st, pos, dloc, ea, split, DUMP, retry=6):
    """Place edges into the stream. Paired chunks: pair i occupies edge slots
    (p, 2t) and (p, 2t+1) of the chunk; didx entry = pair id. Returns
    (gidx, didx, eaT)."""
    gidx = np.zeros(st.ESH, np.int16)
    didx = np.zeros(st.DSH, np.int16)
    eaT = np.zeros((ED + 1, st.ESH), np.float32)
    off = doff = 0
    for k, CH, tb, kind in st.groups:
        base = 0 if tb == 0 else split
        a, b, sng = _split_group(pos, dloc, split, tb)
        if kind == "p":
            didx[doff:doff + k * CH // 2] = DUMP // 2
            if len(a):
                pids = dloc[a] // 2
                order, cid = _assign_chunks(pids, k, CH // 2)
                for ki in range(k):
                    m = order[cid == ki]
                    npair = len(m)
                    e0 = off + ki * CH
                    d0 = doff + ki * (CH // 2)
                    # pair j -> partition j%128, blocks 2*(j//128), +1
                    j = np.arange(npair)
                    sa = e0 + (j // P) * 2 * P + (j % P)
                    sb = sa + P
                    ia, ib = a[m], b[m]
                    gidx[sa] = (pos[ia] - base).astype(np.int16)
                    gidx[sb] = (pos[ib] - base).astype(np.int16)
                    eaT[:ED, sa] = ea[ia].T
                    eaT[:ED, sb] = ea[ib].T
                    eaT[ED, sa] = 1.0
                    eaT[ED, sb] = 1.0
                    didx[d0:d0 + npair] = pids[m].astype(np.int16)
            off += k * CH
            doff += k * (CH // 2)
        else:
            didx[doff:doff + k * CH] = DUMP
            if len(sng):
                order, cid = _assign_chunks(dloc[sng], k, CH)
                for ki in range(k):
                    ke = sng[order[cid == ki]]
                    e0 = off + ki * CH
                    d0 = doff + ki * CH
                    nk = len(ke)
                    gidx[e0:e0 + nk] = (pos[ke] - base).astype(np.int16)
                    didx[d0:d0 + nk] = dloc[ke].astype(np.int16)
                    eaT[:ED, e0:e0 + nk] = ea[ke].T
                    eaT[ED, e0:e0 + nk] = 1.0
            off += k * CH
            doff += k * CH
    return gidx, didx, eaT


def _preprocess(x, edge_index, edge_attr, batch, G=4096, CHUNK_MAX=6400,
                TAB0=32768):
    src = np.asarray(edge_index[0], np.int64)
    dst = np.asarray(edge_index[1], np.int64)
    batch = np.asarray(batch, np.int64)
    ea = np.asarray(edge_attr, np.float32)
    GS = G // C
    gb = np.searchsorted(batch, np.arange(0, G + 1, GS))
    ncnt = np.diff(gb)
    NCH = 512
    N_SH = int(math.ceil(ncnt.max() / NCH) * NCH)
    owner = np.searchsorted(gb, dst, side="right") - 1

    cores = []
    for c in range(C):
        em = np.nonzero(owner == c)[0]
        s_c, d_c = src[em], dst[em]
        uniq, inv = np.unique(s_c, return_inverse=True)
        cores.append(dict(em=em, uniq=uniq, inv=inv, dloc=d_c - gb[c],
                          ea=ea[em]))
    max_m = max(len(pc["uniq"]) for pc in cores)
    assert max_m <= TAB0 + 32768
    TAB1 = int(math.ceil(max(max_m - TAB0, 128) / 128) * 128)

    # A2A block size: rows core c needs from owner o
    need = np.zeros((C, C), np.int64)
    for c in range(C):
        own = np.searchsorted(gb, cores[c]["uniq"], side="right") - 1
        cores[c]["uniq_owner"] = own
        for o in range(C):
            need[c, o] = int((own == o).sum())
    B = int(math.ceil((need.max() + 1) / 128) * 128)
    assert C * B <= TAB0 + 32768, f"A2A table too large: {C * B}"

    # L2 table position per uniq row: block(owner)*B + rank within block
    for c in range(C):
        own = cores[c]["uniq_owner"]
        r = np.zeros(len(own), np.int64)
        for o in range(C):
            m = own == o
            r[m] = np.arange(m.sum())
        cores[c]["pos2"] = (own * B + r)[cores[c]["inv"]]  # per-edge

    s1 = _mk_stream([pc["inv"] for pc in cores],
                    [pc["dloc"] for pc in cores],
                    None, TAB0, CHUNK_MAX)
    s2 = _mk_stream([pc["pos2"] for pc in cores],
                    [pc["dloc"] for pc in cores],
                    None, TAB0, CHUNK_MAX)

    GSP = max(P, int(math.ceil(GS / P) * P))
    cfg = CFG(TAB0=TAB0, TAB1=TAB1, s1=s1, s2=s2, N_SH=N_SH, B=B,
              NCH=NCH, GS=GS, GSP=GSP)

    DUMP = N_SH
    per_core = []
    for c in range(C):
        pc = cores[c]
        g1 = _fill_stream(s1, pc["inv"], pc["dloc"], pc["ea"], TAB0, DUMP)
        g2 = _fill_stream(s2, pc["pos2"], pc["dloc"], pc["ea"], TAB0, DUMP)

        n_c = ncnt[c]
        xT = np.zeros((ND, N_SH), np.float32)
        xT[:, :n_c] = np.asarray(x)[gb[c]:gb[c + 1]].T

        # a2a send-side: rows this core must send to each dest d = the local
        # node ids of x-rows dest d needs from us
        sg = np.zeros(C * B, np.int16)  # filled below (needs other cores)

        # pooling structures
        bl = batch[gb[c]:gb[c + 1]] - c * GS
        blp = np.full(N_SH, -1, np.int64)
        blp[:n_c] = bl
        NT = N_SH // P
        tiles = blp.reshape(NT, P)
        g_first = np.array([t[t >= 0].min() if (t >= 0).any() else 0
                            for t in tiles])
        relg = np.where(blp >= 0, blp - np.repeat(g_first, P), 255.0)
        cnt = np.bincount(bl, minlength=GS).astype(np.float32)
        gstart = np.searchsorted(bl, np.arange(GS))
        gend = np.searchsorted(bl, np.arange(GS), side="right")
        t_lo, t_hi = gstart // P, np.maximum(gend - 1, gstart) // P

        per_core.append(dict(
            gidx1=g1[0], didx1=g1[1], eaT1=g1[2],
            gidx2=g2[0], didx2=g2[1], eaT2=g2[2],
            xT=xT, uniq=pc["uniq"], uniq_owner=pc["uniq_owner"], n_c=n_c,
            relg=relg.astype(np.float32), g_first=g_first, cnt=cnt,
            t_lo=t_lo, t_hi=t_hi, sg=sg,
        ))

    # send-side gather indices: core o sends to dest c the rows c needs from o
    for o in range(C):
        sg = np.zeros(C * B, np.int16)
        for c in range(C):
            m = per_core[c]["uniq_owner"] == o
            rows = per_core[c]["uniq"][m] - gb[o]   # local node idx on o
            sg[c * B:c * B + len(rows)] = rows.astype(np.int16)
        per_core[o]["sg"] = sg

    cfg.GW = int(max((pc["relg"][pc["relg"] != 255.0]).max() + 1
                     if (pc["relg"] != 255.0).any() else 1 for pc in per_core))
    cfg.PG = int(max((pc["t_hi"] - pc["t_lo"] + 1)[pc["cnt"] > 0].max()
                     if (pc["cnt"] > 0).any() else 1 for pc in per_core))
    cfg.n_pool_idx = int(math.ceil(cfg.PG * cfg.GSP / 128) * 128)

    NT = cfg.NT
    ZPAD = NT * cfg.GW
    for pc in per_core:
        pidx = np.full(cfg.n_pool_idx, ZPAD, np.int16)
        for g in range(GS):
            if pc["cnt"][g] <= 0:
                continue
            for p, t in enumerate(range(pc["t_lo"][g], pc["t_hi"][g] + 1)):
                rel = g - pc["g_first"][t]
                pidx[p * cfg.GSP + g] = t * cfg.GW + rel
        pc["pool_idx"] = pidx
        pc["cnt_gm"] = np.maximum(
            np.pad(pc["cnt"], (0, cfg.GSP - GS)), 1.0
        ).reshape(cfg.GSP // P, P).T.astype(np.float32)

    relids = np.tile(np.arange(cfg.GW, dtype=np.float32), (P, 1))
    return cfg, gb, per_core, relids


def _gather_tables(cfg, per_core, x):
    out = []
    for pc in per_core:
        uniq = pc["uniq"]
        t0 = np.zeros((cfg.TAB0, ND), np.float32)
        t1 = np.zeros((cfg.TAB1, ND), np.float32)
        n0 = min(len(uniq), cfg.TAB0)
        t0[:n0] = x[uniq[:n0]]
        if len(uniq) > cfg.TAB0:
            t1[:len(uniq) - cfg.TAB0] = x[uniq[cfg.TAB0:]]
        out.append((t0, t1))
    return out


def _edge_phase(ctx, tc, nc, st, tabs, gidx_d, didx_d, eaT_d, w_e, acc_aps, tag):
    gp = ctx.enter_context(tc.tile_pool(name=f"eg{tag}", bufs=2))
    xp = ctx.enter_context(tc.tile_pool(name=f"ex{tag}", bufs=2))
    ep = ctx.enter_context(tc.tile_pool(name=f"ee{tag}", bufs=2))
    dp = ctx.enter_context(tc.tile_pool(name=f"ed{tag}", bufs=2))
    mp = ctx.enter_context(tc.tile_pool(name=f"em{tag}", bufs=2))
    pp = ctx.enter_context(tc.tile_pool(name=f"ep{tag}", bufs=2, space="PSUM"))
    for ci, (off, CH, tb, kind, doff) in enumerate(st.chunks):
        KB = CH // P
        gix = gp.tile([P, CH // 16], I16, tag="gix")
        nc.sync.dma_start(gix[:], gidx_d[:, off // 16:(off + CH) // 16])
        xg = xp.tile([P, KB * ND], F32, tag="xg")
        nc.gpsimd.dma_gather(
            out_ap=xg[:].rearrange("p (k e) -> p k e", e=ND),
            in_ap=tabs[tb], idxs_ap=gix[:],
            num_idxs=CH, num_idxs_reg=CH, elem_size=ND, single_packet=False)
        eat = ep.tile([ED + 1, CH], F32, tag="eat")
        nc.sync.dma_start(eat[:], eaT_d[:, off:off + CH])
        ND_IDX = CH if kind == "s" else CH // 2
        dix = dp.tile([P, ND_IDX // 16], I16, tag="dix")
        nc.sync.dma_start(dix[:], didx_d[:, doff // 16:(doff + ND_IDX) // 16])
        msg = mp.tile([P, KB * ND], F32, tag="msg")
        for g8 in range(0, KB, 8):
            nb = min(8, KB - g8)
            ps = pp.tile([P, 512], F32, tag="lin")
            for j in range(nb):
                b = g8 + j
                nc.tensor.matmul(out=ps[:, j * ND:(j + 1) * ND],
                                 lhsT=eat[:, b * P:(b + 1) * P],
                                 rhs=w_e[:], start=True, stop=True)
            sl = slice(g8 * ND, (g8 + nb) * ND)
            nc.vector.tensor_add(out=msg[:, sl], in0=xg[:, sl],
                                 in1=ps[:, :nb * ND])
            nc.scalar.activation(out=msg[:, sl], in_=msg[:, sl], func=AF.Relu)
        if kind == "s":
            nc.gpsimd.dma_scatter_add(
                out_ap=acc_aps[0],
                in_ap=msg[:].rearrange("p (k e) -> p k e", e=ND),
                idxs_ap=dix[:], num_idxs=CH, num_idxs_reg=CH, elem_size=ND,
                single_packet=False)
        else:
            nc.gpsimd.dma_scatter_add(
                out_ap=acc_aps[0].rearrange("(m two) e -> m (two e)", two=2),
                in_ap=msg[:].rearrange("p (k e) -> p k e", e=2 * ND),
                idxs_ap=dix[:], num_idxs=CH // 2, num_idxs_reg=CH // 2,
                elem_size=2 * ND, single_packet=False)


def _zero_dram_rows(nc, t, rows, cols, zt):
    RB = 2048
    for r0 in range(0, rows, RB):
        rb = min(RB, rows - r0)
        nc.sync.dma_start(
            out=t[r0:r0 + rb, :].rearrange("(p r) e -> p (r e)", p=P),
            in_=zt[:, :rb * cols // P])


def _node_mlp(ctx, tc, nc, cfg, accs, xT_d, ident, w1, b1, w2, b2, HID,
              outT_d, last_relu, out_sbuf_cb=None, rows_cb=None):
    NCH = cfg.NCH
    ap = ctx.enter_context(tc.tile_pool(name="np_acc", bufs=3))
    xp = ctx.enter_context(tc.tile_pool(name="np_x", bufs=2))
    hp = ctx.enter_context(tc.tile_pool(name="np_h", bufs=2))
    zp = ctx.enter_context(tc.tile_pool(name="np_z", bufs=2))
    op = ctx.enter_context(tc.tile_pool(name="np_o", bufs=2))
    tp = ctx.enter_context(tc.tile_pool(name="np_tp", bufs=2, space="PSUM"))
    mp = ctx.enter_context(tc.tile_pool(name="np_mm", bufs=1, space="PSUM"))
    rp = ctx.enter_context(tc.tile_pool(name="np_r", bufs=2))

    HID2 = w2.shape[1]
    for t in range(cfg.N_SH // NCH):
        xT = xp.tile([ND, NCH], F32)
        nc.sync.dma_start(xT[:], xT_d[:, t * NCH:(t + 1) * NCH])
        at = ap.tile([P, NCH // P * ND], F32)
        nc.sync.dma_start(
            at[:].rearrange("p (j e) -> p j e", e=ND),
            accs[0][t * NCH:(t + 1) * NCH, :].rearrange("(j p) e -> p j e", p=P))
        hT = hp.tile([ND, NCH], F32)
        for j in range(NCH // P):
            pt = tp.tile([ND, P], F32, tag="tp")
            nc.tensor.transpose(out=pt[:], in_=at[:, j * ND:(j + 1) * ND],
                                identity=ident[:])
            nc.vector.tensor_add(out=hT[:, j * P:(j + 1) * P],
                                 in0=pt[:], in1=xT[:, j * P:(j + 1) * P])
        z1p = mp.tile([HID, NCH], F32, tag="mm1")
        nc.tensor.matmul(out=z1p[:], lhsT=w1[:], rhs=hT[:], start=True, stop=True)
        z1 = zp.tile([HID, NCH], F32)
        nc.scalar.activation(out=z1[:], in_=z1p[:], func=AF.Relu, bias=b1[:])
        z2p = mp.tile([HID2, NCH], F32, tag="mm2")
        nc.tensor.matmul(out=z2p[:], lhsT=w2[:], rhs=z1[:], start=True, stop=True)
        o = op.tile([HID2, NCH], F32)
        nc.scalar.activation(out=o[:], in_=z2p[:],
                             func=AF.Relu if last_relu else AF.Identity,
                             bias=b2[:])
        if outT_d is not None:
            nc.sync.dma_start(out=outT_d[:, t * NCH:(t + 1) * NCH], in_=o[:])
        if rows_cb is not None:
            # also produce node-major rows (transpose o back)
            rt = rp.tile([P, NCH // P * HID2], F32)
            for j in range(NCH // P):
                pt2 = tp.tile([P, HID2], F32, tag="tp2")
                nc.tensor.transpose(out=pt2[:], in_=o[:, j * P:(j + 1) * P],
                                    identity=ident[:HID2, :HID2])
                nc.vector.tensor_copy(out=rt[:, j * HID2:(j + 1) * HID2],
                                      in_=pt2[:])
            rows_cb(t, rt)
        if out_sbuf_cb is not None:
            out_sbuf_cb(t, o)


def _build(cfg):
    nc = bacc.Bacc("TRN2", target_bir_lowering=False, debug=False,
                   num_devices=C)
    d = {}

    def inp(name, shape, dt=F32):
        d[name] = nc.dram_tensor(name, shape, dt, kind="ExternalInput").ap()

    inp("tab0", [cfg.TAB0, ND]); inp("tab1", [cfg.TAB1, ND])
    inp("gidx1", [P, cfg.s1.ESH // 16], I16); inp("didx1", [P, cfg.s1.DSH // 16], I16)
    inp("eaT1", [ED + 1, cfg.s1.ESH])
    inp("gidx2", [P, cfg.s2.ESH // 16], I16); inp("didx2", [P, cfg.s2.DSH // 16], I16)
    inp("eaT2", [ED + 1, cfg.s2.ESH])
    inp("xT", [ND, cfg.N_SH])
    inp("sg", [P, C * cfg.B // 16], I16)
    inp("w_e1", [ED + 1, ND]); inp("w11", [ND, ND]); inp("b11", [ND, 1])
    inp("w12", [ND, ND]); inp("b12", [ND, 1])
    inp("w_e2", [ED + 1, ND]); inp("w21", [ND, EMB]); inp("b21", [EMB, 1])
    inp("w22", [EMB, EMB]); inp("b22", [EMB, 1])
    inp("relg", [P, cfg.NT]); inp("relids", [P, cfg.GW])
    inp("pool_idx", [P, cfg.n_pool_idx // 16], I16)
    inp("cnt_gm", [P, cfg.GSP // P]); inp("usrT", [USR, cfg.GSP])
    for nm, shp in (("hw1a", [EMB, 128]), ("hw1b", [USR, 128]), ("hb1", [128, 1]),
                    ("hw2", [128, 64]), ("hb2", [64, 1]), ("hw3", [64, 32]),
                    ("hb3", [32, 1]), ("hw4", [32, 16]), ("hb4", [16, 1]),
                    ("hw5", [16, 1]), ("hb5", [1, 1])):
        inp(nm, shp)
    yT = nc.dram_tensor("yT", [1, cfg.GSP], F32, kind="ExternalOutput").ap()

    GW, PG, NT, GSP, B = cfg.GW, cfg.PG, cfg.NT, cfg.GSP, cfg.B
    NROW = NT * GW + P

    with tile.TileContext(nc) as tc, ExitStack() as ctx:
        const = ctx.enter_context(tc.tile_pool(name="const", bufs=1))

        def ld(name, shape):
            t = const.tile(shape, F32, name=f"c_{name}")
            nc.sync.dma_start(t[:], d[name])
            return t

        w_e1 = ld("w_e1", [ED + 1, ND])
        w11 = ld("w11", [ND, ND]); b11 = ld("b11", [ND, 1])
        w12 = ld("w12", [ND, ND]); b12 = ld("b12", [ND, 1])
        w_e2 = ld("w_e2", [ED + 1, ND])
        w21 = ld("w21", [ND, EMB]); b21 = ld("b21", [EMB, 1])
        w22 = ld("w22", [EMB, EMB]); b22 = ld("b22", [EMB, 1])
        relg = ld("relg", [P, cfg.NT])
        relids = ld("relids", [P, GW])
        ident = const.tile([P, P], F32, name="ident")
        make_identity(nc, ident[:])
        zt = const.tile([P, 1024], F32, name="zt")
        nc.vector.memset(zt[:], 0.0)

        dram = ctx.enter_context(tc.tile_pool(name="dram", bufs=1, space="DRAM"))
        acc1 = dram.tile([cfg.N_SH + P, ND], F32)
        acc2 = dram.tile([cfg.N_SH + P, ND], F32)
        h1T = dram.tile([ND, cfg.N_SH], F32)
        h1r = dram.tile([cfg.N_SH, ND], F32)
        a2a_in = dram.tile([C * B, ND], F32)
        a2a_out = dram.tile([C * B, ND], F32)
        parts = dram.tile([NROW, P], F32)
        _zero_dram_rows(nc, acc1, cfg.N_SH + P, ND, zt)
        _zero_dram_rows(nc, acc2, cfg.N_SH + P, ND, zt)
        nc.sync.dma_start(
            out=parts[NT * GW:NT * GW + P, :].rearrange("(p r) e -> p (r e)", p=P),
            in_=zt[:, :P])

        # ---- layer 1 edges ----
        with ExitStack() as ectx:
            _edge_phase(ectx, tc, nc, cfg.s1, (d["tab0"], d["tab1"]),
                        d["gidx1"], d["didx1"], d["eaT1"], w_e1,
                        (acc1[:],), "1")

        # ---- layer 1 nodes (h1T + h1 rows) ----
        def rows_cb(t, rt):
            nc.sync.dma_start(
                out=h1r[t * cfg.NCH:(t + 1) * cfg.NCH, :]
                .rearrange("(j p) e -> p j e", p=P),
                in_=rt[:].rearrange("p (j e) -> p j e", e=ND))

        with ExitStack() as nctx:
            _node_mlp(nctx, tc, nc, cfg, (acc1,), d["xT"], ident,
                      w11, b11, w12, b12, ND, h1T[:], last_relu=True,
                      rows_cb=rows_cb)

        # ---- exchange: pre-gather + AllToAll ----
        with ExitStack() as actx:
            agp = actx.enter_context(tc.tile_pool(name="a2a", bufs=2))
            sgp = actx.enter_context(tc.tile_pool(name="a2as", bufs=2))
            for dest in range(C):
                six = sgp.tile([P, B // 16], I16, tag="six")
                nc.sync.dma_start(six[:], d["sg"][:, dest * B // 16:(dest + 1) * B // 16])
                gt = agp.tile([P, B // P * ND], F32, tag="gt")
                nc.gpsimd.dma_gather(
                    out_ap=gt[:].rearrange("p (k e) -> p k e", e=ND),
                    in_ap=h1r[:], idxs_ap=six[:],
                    num_idxs=B, num_idxs_reg=B, elem_size=ND,
                    single_packet=False)
                nc.sync.dma_start(
                    out=a2a_in[dest * B:(dest + 1) * B, :]
                    .rearrange("(k p) e -> p k e", p=P),
                    in_=gt[:].rearrange("p (k e) -> p k e", e=ND))
            nc.gpsimd.collective_compute(
                "AllToAll", mybir.AluOpType.bypass,
                replica_groups=[list(range(C))],
                ins=[a2a_in[:].opt()], outs=[a2a_out[:].opt()])

        # ---- layer 2 edges (tables = a2a_out split at TAB0) ----
        t0hi = min(cfg.TAB0, C * B)
        t2_0 = a2a_out[:t0hi, :]
        t2_1 = a2a_out[t0hi:, :] if C * B > cfg.TAB0 else t2_0
        with ExitStack() as ectx:
            _edge_phase(ectx, tc, nc, cfg.s2, (t2_0, t2_1),
                        d["gidx2"], d["didx2"], d["eaT2"], w_e2,
                        (acc2[:],), "2")

        # ---- layer 2 nodes + pooling partials ----
        with ExitStack() as nctx:
            pool_sb = nctx.enter_context(tc.tile_pool(name="pl_sb", bufs=2))
            pool_ps = nctx.enter_context(tc.tile_pool(name="pl_ps", bufs=1, space="PSUM"))
            pool_s = nctx.enter_context(tc.tile_pool(name="pl_s", bufs=2))
            pool_nm = nctx.enter_context(tc.tile_pool(name="pl_nm", bufs=2))

            def pool_cb(t, embT):
                npart = cfg.NCH // P
                S4 = pool_s.tile([P, npart * GW], F32, tag="S4")
                nc.vector.tensor_tensor(
                    out=S4[:].rearrange("p (j g) -> p j g", g=GW),
                    in0=relg[:, t * npart:(t + 1) * npart]
                    .unsqueeze(2).broadcast_to([P, npart, GW]),
                    in1=relids[:].unsqueeze(1).broadcast_to([P, npart, GW]),
                    op=ALU.is_equal)
                for j in range(npart):
                    tl = t * npart + j
                    tps = pool_ps.tile([P, P], F32, tag="tpose")
                    nc.tensor.transpose(out=tps[:], in_=embT[:, j * P:(j + 1) * P],
                                        identity=ident[:])
                    enm = pool_nm.tile([P, P], F32)
                    nc.vector.tensor_copy(out=enm[:], in_=tps[:])
                    pps = pool_ps.tile([GW, P], F32, tag="part")
                    nc.tensor.matmul(out=pps[:], lhsT=S4[:, j * GW:(j + 1) * GW],
                                     rhs=enm[:], start=True, stop=True)
                    psb = pool_sb.tile([GW, P], F32)
                    nc.vector.tensor_copy(out=psb[:], in_=pps[:])
                    nc.sync.dma_start(out=parts[tl * GW:(tl + 1) * GW, :],
                                      in_=psb[:])

            _node_mlp(nctx, tc, nc, cfg, (acc2,), h1T[:], ident,
                      w21, b21, w22, b22, EMB, None, last_relu=False,
                      out_sbuf_cb=pool_cb)

        # ---- pool reduce + head ----
        with ExitStack() as hctx:
            hp = hctx.enter_context(tc.tile_pool(name="hd", bufs=1))
            hps = hctx.enter_context(tc.tile_pool(name="hd_ps", bufs=2, space="PSUM"))
            pix = hp.tile([P, cfg.n_pool_idx // 16], I16)
            nc.sync.dma_start(pix[:], d["pool_idx"])
            NPB = cfg.n_pool_idx // P
            gpo = hp.tile([P, NPB * P], F32)
            nc.gpsimd.dma_gather(
                out_ap=gpo[:].rearrange("p (k e) -> p k e", e=P),
                in_ap=parts[:], idxs_ap=pix[:],
                num_idxs=cfg.n_pool_idx, num_idxs_reg=cfg.n_pool_idx,
                elem_size=P, single_packet=False)
            GB = GSP // P
            v = gpo[:].rearrange("p (q b e) -> p q b e", q=PG, b=GB)
            pooled = hp.tile([P, GB * P], F32)
            pv = pooled[:].rearrange("p (b e) -> p b e", b=GB)
            if PG == 1:
                nc.vector.tensor_copy(out=pv, in_=v[:, 0])
            else:
                nc.vector.tensor_add(out=pv, in0=v[:, 0], in1=v[:, 1])
                for q in range(2, PG):
                    nc.vector.tensor_add(out=pv, in0=pv, in1=v[:, q])
            cntg = hp.tile([P, GB], F32)
            nc.sync.dma_start(cntg[:], d["cnt_gm"])
            invc = hp.tile([P, GB], F32)
            nc.vector.reciprocal(invc[:], cntg[:])
            for b in range(GB):
                nc.vector.tensor_tensor(
                    out=pooled[:, b * P:(b + 1) * P],
                    in0=pooled[:, b * P:(b + 1) * P],
                    in1=invc[:, b:b + 1].to_broadcast([P, P]), op=ALU.mult)
            embT = hp.tile([P, GSP], F32)
            for b in range(GB):
                tps = hps.tile([P, P], F32, tag="hd")
                nc.tensor.transpose(out=tps[:], in_=pooled[:, b * P:(b + 1) * P],
                                    identity=ident[:])
                nc.vector.tensor_copy(out=embT[:, b * P:(b + 1) * P], in_=tps[:])
            usrT = hp.tile([USR, GSP], F32)
            nc.sync.dma_start(usrT[:], d["usrT"])
            hw = {nm: hp.tile(d[nm].shape, F32, name=f"t_{nm}")
                  for nm in ("hw1a", "hw1b", "hb1", "hw2", "hb2", "hw3", "hb3",
                             "hw4", "hb4", "hw5", "hb5")}
            for nm, t in hw.items():
                nc.sync.dma_start(t[:], d[nm])
            z1p = hps.tile([128, GSP], F32, tag="hd")
            nc.tensor.matmul(out=z1p[:], lhsT=hw["hw1a"][:], rhs=embT[:],
                             start=True, stop=False)
            nc.tensor.matmul(out=z1p[:], lhsT=hw["hw1b"][:], rhs=usrT[:],
                             start=False, stop=True)
            z1 = hp.tile([128, GSP], F32)
            nc.scalar.activation(out=z1[:], in_=z1p[:], func=AF.Relu, bias=hw["hb1"][:])
            z2p = hps.tile([64, GSP], F32, tag="hd")
            nc.tensor.matmul(out=z2p[:], lhsT=hw["hw2"][:], rhs=z1[:], start=True, stop=True)
            z2 = hp.tile([64, GSP], F32)
            nc.scalar.activation(out=z2[:], in_=z2p[:], func=AF.Relu, bias=hw["hb2"][:])
            z3p = hps.tile([32, GSP], F32, tag="hd")
            nc.tensor.matmul(out=z3p[:], lhsT=hw["hw3"][:], rhs=z2[:], start=True, stop=True)
            z3 = hp.tile([32, GSP], F32)
            nc.scalar.activation(out=z3[:], in_=z3p[:], func=AF.Relu, bias=hw["hb3"][:])
            z4p = hps.tile([16, GSP], F32, tag="hd")
            nc.tensor.matmul(out=z4p[:], lhsT=hw["hw4"][:], rhs=z3[:], start=True, stop=True)
            z4 = hp.tile([16, GSP], F32)
            nc.scalar.activation(out=z4[:], in_=z4p[:], func=AF.Relu, bias=hw["hb4"][:])
            z5p = hps.tile([1, GSP], F32, tag="hd")
            nc.tensor.matmul(out=z5p[:], lhsT=hw["hw5"][:], rhs=z4[:], start=True, stop=True)
            z5 = hp.tile([1, GSP], F32)
            nc.scalar.activation(out=z5[:], in_=z5p[:], func=AF.Identity, bias=hw["hb5"][:])
            nc.sync.dma_start(out=yT, in_=z5[:])

    nc.compile()
    return nc


def _make_in_maps(cfg, per_core, inputs, relids):
    f32 = lambda a: np.ascontiguousarray(np.asarray(a, np.float32))
    x = f32(inputs["x"])
    usr = f32(inputs["usr"])
    tabs = _gather_tables(cfg, per_core, x)
    w_e1 = np.vstack([f32(inputs["e1_w"]), f32(inputs["e1_b"])[None, :]])
    w_e2 = np.vstack([f32(inputs["e2_w"]), f32(inputs["e2_b"])[None, :]])
    NT = cfg.NT
    in_maps = []
    for c, pc in enumerate(per_core):
        usrT = np.zeros((USR, cfg.GSP), np.float32)
        usrT[:, :cfg.GS] = usr[c * cfg.GS:(c + 1) * cfg.GS].T
        in_maps.append(dict(
            tab0=tabs[c][0], tab1=tabs[c][1],
            gidx1=_wrap16(pc["gidx1"]), didx1=_wrap16(pc["didx1"]),
            eaT1=np.ascontiguousarray(pc["eaT1"]),
            gidx2=_wrap16(pc["gidx2"]), didx2=_wrap16(pc["didx2"]),
            eaT2=np.ascontiguousarray(pc["eaT2"]),
            xT=pc["xT"], sg=_wrap16(pc["sg"]),
            w_e1=w_e1, w11=f32(inputs["n1_w1"]), b11=f32(inputs["n1_b1"])[:, None],
            w12=f32(inputs["n1_w2"]), b12=f32(inputs["n1_b2"])[:, None],
            w_e2=w_e2, w21=f32(inputs["n2_w1"]), b21=f32(inputs["n2_b1"])[:, None],
            w22=f32(inputs["n2_w2"]), b22=f32(inputs["n2_b2"])[:, None],
            relg=np.ascontiguousarray(pc["relg"].reshape(NT, P).T),
            relids=relids, pool_idx=_wrap16(pc["pool_idx"]),
            cnt_gm=pc["cnt_gm"], usrT=usrT,
            hw1a=f32(inputs["h1_w"])[:EMB], hw1b=f32(inputs["h1_w"])[EMB:],
            hb1=f32(inputs["h1_b"])[:, None],
            hw2=f32(inputs["h2_w"]), hb2=f32(inputs["h2_b"])[:, None],
            hw3=f32(inputs["h3_w"]), hb3=f32(inputs["h3_b"])[:, None],
            hw4=f32(inputs["h4_w"]), hb4=f32(inputs["h4_b"])[:, None],
            hw5=f32(inputs["h5_w"]), hb5=f32(inputs["h5_b"])[:, None]))
    return in_maps


def kernel(**inputs):
    cfg, gb, per_core, relids = _preprocess(
        np.asarray(inputs["x"], np.float32), inputs["edge_index"],
        np.asarray(inputs["edge_attr"], np.float32), inputs["batch"])
    nc = _build(cfg)
    in_maps = _make_in_maps(cfg, per_core, inputs, relids)
    res = bass_utils.run_bass_kernel_spmd(nc, in_maps, core_ids=list(range(C)))
    out = np.concatenate([res.results[c]["yT"][0, :cfg.GS] for c in range(C)])
    kernel._last = res
    return out[:, None].astype(np.float32)



# revision 2
# speedup vs baseline: 1.0668x; 1.0668x over previous
"""Trainium2 Bass kernel v2 for nn_DockingTimeModel (2-layer GINE GNN +
mean-pool + MLP head), single merged SPMD launch on 8 NeuronCores.

Design (vs v1 baseline): eliminate all HBM dma_scatter_add and HBM dma_gather.
Edges are sorted by dst and grouped per 128-node tile (tiles processed in
edge-count-sorted order so the per-position edge-tile count kseq is shared
across cores). Segment-sum is computed on the PE as indicator matmuls:
msg^T[128e,64] @ I[128e,128n] accumulated in PSUM per node tile. Neighbor
features are fetched with SBUF-source dma_gather (fp16 tables resident in
SBUF; 256B stripe rows), which avoids the small-packet HBM penalty entirely.
Layer->layer exchange: h1 rows kept in an SBUF row-table, pre-gathered per
(dest,half) and exchanged with one AllToAll in fp16 feat-major blocks; the
receive side rebuilds SBUF gather tables via PE transposes. Pooling and the
MLP head follow the v1 structure (indicator matmuls + one 512B-row gather).
"""
import sys

sys.path.insert(0, "/opt/trn_rl_repo")

import math
from contextlib import ExitStack

import numpy as np

from concourse import bacc, bass, mybir, tile
from concourse import bass_utils
from concourse.masks import make_identity

F32 = mybir.dt.float32
F16 = mybir.dt.float16
I16 = mybir.dt.int16
AF = mybir.ActivationFunctionType
ALU = mybir.AluOpType

C = 8
P = 128
ND = 64
ED = 16
EMB = 128
USR = 12
G = 4096
GS = G // C
GSP = GS  # 512, multiple of 128
CT = 9    # node tiles per chunk


def _wrap16(idx):
    L = len(idx)
    assert L % 16 == 0
    a = np.asarray(idx, np.int16).reshape(L // 16, 16).T
    return np.tile(a, (8, 1))


def _pad128(n):
    return int(math.ceil(max(n, 1) / 128) * 128)


class CFG:
    pass


def _preprocess(x, edge_index, edge_attr, batch):
    src = np.asarray(edge_index[0], np.int64)
    dst = np.asarray(edge_index[1], np.int64)
    batch = np.asarray(batch, np.int64)
    ea = np.asarray(edge_attr, np.float32)
    x = np.asarray(x, np.float32)

    gb = np.searchsorted(batch, np.arange(0, G + 1, GS))
    ncnt = np.diff(gb)
    NT = int(math.ceil(ncnt.max() / P))
    N_SH = NT * P
    HALF = NT // 2
    owner = np.searchsorted(gb, dst, side="right") - 1

    cores = []
    k_all = []
    for c in range(C):
        em = np.nonzero(owner == c)[0]
        d_loc = dst[em] - gb[c]
        t_phys = d_loc // P
        cnt_tile = np.bincount(t_phys, minlength=NT)
        perm = np.argsort(-cnt_tile, kind="stable")
        posof = np.empty(NT, np.int64)
        posof[perm] = np.arange(NT)
        k_core = np.maximum(np.ceil(cnt_tile[perm] / P).astype(np.int64), 1)
        k_all.append(k_core)
        cores.append(dict(em=em, d_loc=d_loc, t_phys=t_phys, perm=perm,
                          posof=posof, n_c=ncnt[c]))
    kseq = np.stack(k_all).max(axis=0)
    offs = np.concatenate([[0], np.cumsum(kseq)]) * P
    ESH = int(offs[-1])

    cfg = CFG()
    cfg.NT, cfg.N_SH, cfg.HALF, cfg.ESH = NT, N_SH, HALF, ESH
    cfg.kseq, cfg.offs = kseq, offs

    # ---- edge slots ----
    for c in range(C):
        pc = cores[c]
        em, d_loc, t_phys, perm = pc["em"], pc["d_loc"], pc["t_phys"], pc["perm"]
        s_c = src[em]
        slot_src = np.zeros(ESH, np.int64)    # global src id per slot
        slot_real = np.zeros(ESH, bool)
        dstrel = np.full(ESH, 255.0, np.float32)
        eaT = np.zeros((ED + 1, ESH), np.float32)
        for i in range(NT):
            T = perm[i]
            el = np.nonzero(t_phys == T)[0]
            n_e = len(el)
            assert n_e <= kseq[i] * P
            s0 = offs[i]
            slot_src[s0:s0 + n_e] = s_c[el]
            slot_real[s0:s0 + n_e] = True
            dstrel[s0:s0 + n_e] = (d_loc[el] - T * P).astype(np.float32)
            eaT[:ED, s0:s0 + n_e] = ea[em[el]].T
            eaT[ED, s0:s0 + n_e] = 1.0
        pc["slot_src"] = slot_src
        pc["slot_real"] = slot_real
        pc["dstrel"] = dstrel
        pc["eaT"] = eaT
        # L1: fused edge stream [81, ESH] = [ea(16); ones(1); x_src(64)]
        es = np.zeros((ED + 1 + ND, ESH), np.float16)
        es[:ED + 1] = eaT.astype(np.float16)
        es[ED + 1:][:, slot_real] = x[slot_src[slot_real]].T.astype(np.float16)
        pc["estream"] = es
        half_of_slot = np.zeros(ESH, np.int64)
        half_of_slot[offs[HALF]:] = 1
        pc["half_of_slot"] = half_of_slot

    # ---- L2 exchange: unique srcs per (receiver, half) split by owner,
    # sub-ordered by the owner's h1 segment so send gathers can fire early ----
    SEG = 4
    segb = [int(round(NT * q / SEG)) for q in range(SEG + 1)]  # tile bounds
    cfg.SEG, cfg.segb = SEG, segb

    def _seg_of_proc(proc):
        return np.searchsorted(segb, proc // P, side="right") - 1

    cntq = np.zeros(SEG, np.int64)
    for c in range(C):
        pc = cores[c]
        for h, name in ((0, "A"), (1, "B")):
            m = pc["slot_real"] & (pc["half_of_slot"] == h)
            u = np.unique(pc["slot_src"][m])
            ou = np.searchsorted(gb, u, side="right") - 1
            # per (owner, seg) counts; order rows by (owner, seg, id)
            useg = np.zeros(len(u), np.int64)
            for o in range(C):
                mo = ou == o
                lo_ = u[mo] - gb[o]
                po = cores[o]
                proc = po["posof"][lo_ // P] * P + (lo_ % P)
                useg[mo] = _seg_of_proc(proc)
                for q in range(SEG):
                    cntq[q] = max(cntq[q], int((useg[mo] == q).sum()))
            pc["uniq2" + name] = u
            pc["uniq2o" + name] = ou
            pc["uniq2s" + name] = useg
    BQ = [_pad128(int(c_)) for c_ in cntq]
    offQ = np.concatenate([[0], np.cumsum(BQ)])
    B_h = int(offQ[-1])
    assert C * B_h < 32768, B_h
    cfg.B_h, cfg.BQ, cfg.offQ = B_h, BQ, offQ

    for c in range(C):
        pc = cores[c]
        gidx2 = np.zeros(ESH, np.int64)
        for h, name in ((0, "A"), (1, "B")):
            u, ou, us = pc["uniq2" + name], pc["uniq2o" + name], pc["uniq2s" + name]
            pos_u = np.zeros(len(u), np.int64)
            for o in range(C):
                for q in range(SEG):
                    mo = (ou == o) & (us == q)
                    pos_u[mo] = o * B_h + offQ[q] + np.arange(mo.sum())
            m = pc["slot_real"] & (pc["half_of_slot"] == h)
            gidx2[m] = pos_u[np.searchsorted(u, pc["slot_src"][m])]
        pc["gidx2"] = gidx2

    # sender-side: sg[(dest*2+half)*B_h + offQ[q] + rank] = seg-local proc row
    for o in range(C):
        sg = np.zeros(C * 2 * B_h, np.int64)
        po = cores[o]
        for c in range(C):
            for h, name in ((0, "A"), (1, "B")):
                pc = cores[c]
                mo = pc["uniq2o" + name] == o
                rows = pc["uniq2" + name][mo]
                us = pc["uniq2s" + name][mo]
                l = rows - gb[o]
                proc = po["posof"][l // P] * P + (l % P)
                for q in range(SEG):
                    mq = us == q
                    base = (c * 2 + h) * B_h + offQ[q]
                    sg[base:base + mq.sum()] = proc[mq] - segb[q] * P
        cores[o]["sg"] = sg

    # ---- x streams / tables ----
    for c in range(C):
        pc = cores[c]
        n_c = pc["n_c"]
        xr = np.zeros((N_SH, ND), np.float32)
        xr[:n_c] = x[gb[c]:gb[c] + n_c]
        # processing order: node proc row i*128+p = phys row perm[i]*128+p
        xproc = xr.reshape(NT, P, ND)[pc["perm"]].reshape(N_SH, ND)
        pc["xT"] = np.ascontiguousarray(xproc.T)

    # ---- pooling ----
    for c in range(C):
        pc = cores[c]
        n_c = pc["n_c"]
        bl = batch[gb[c]:gb[c] + n_c] - c * GS
        blp = np.full(N_SH, -1, np.int64)
        blp[:n_c] = bl
        blproc = blp.reshape(NT, P)[pc["perm"]]  # [NT, P] proc order
        g_first = np.array([t[t >= 0].min() if (t >= 0).any() else 0
                            for t in blproc])
        relg = np.where(blproc >= 0,
                        blproc - g_first[:, None], 255.0).astype(np.float32)
        pc["relg"] = relg          # [NT, P]
        pc["g_first"] = g_first
        pc["bl"] = bl
        pc["cnt"] = np.bincount(bl, minlength=GS).astype(np.float32)
    GW = int(max((pc["relg"][pc["relg"] != 255.0]).max() + 1 for pc in cores))
    cfg.GW = GW
    PG = 1
    for c in range(C):
        pc = cores[c]
        # proc tiles containing each graph
        tiles_of_g = [[] for _ in range(GS)]
        for i in range(NT):
            gs = np.unique(pc["relg"][i][pc["relg"][i] != 255.0])
            for r in gs:
                tiles_of_g[int(pc["g_first"][i] + r)].append(i)
        pc["tiles_of_g"] = tiles_of_g
        PG = max(PG, max((len(t) for t in tiles_of_g), default=1))
    cfg.PG = PG
    cfg.n_pool_idx = _pad128(PG * GSP)
    ZPAD = NT * GW
    for c in range(C):
        pc = cores[c]
        pidx = np.full(cfg.n_pool_idx, ZPAD, np.int16)
        for g in range(GS):
            for q, t in enumerate(pc["tiles_of_g"][g]):
                rel = g - pc["g_first"][t]
                pidx[q * GSP + g] = t * GW + rel
        pc["pool_idx"] = pidx
        pc["cnt_gm"] = np.maximum(pc["cnt"], 1.0).reshape(GSP // P, P).T.copy()

    # chunk layout: list of (tile_lo, ntiles) per half
    def _chunks(lo, hi):
        out = []
        i = lo
        while i < hi:
            n = min(CT, hi - i)
            out.append((i, n))
            i += n
        return out
    cfg.chunksA = _chunks(0, HALF)
    cfg.chunksB = _chunks(HALF, NT)
    return cfg, gb, cores


def _build(cfg):
    nc = bacc.Bacc("TRN2", target_bir_lowering=False, debug=False,
                   num_devices=C)
    d = {}

    def inp(name, shape, dt=F32):
        d[name] = nc.dram_tensor(name, shape, dt, kind="ExternalInput").ap()

    NT, N_SH, ESH, B_h = cfg.NT, cfg.N_SH, cfg.ESH, cfg.B_h
    GW, PG = cfg.GW, cfg.PG
    inp("estream", [ED + 1 + ND, ESH], F16)
    inp("gidx2", [P, ESH // 16], I16)
    inp("eaT", [ED + 1, ESH], F16)
    inp("dstrel", [P, ESH // P])
    inp("xT", [ND, N_SH])
    inp("sg", [P, C * 2 * B_h // 16], I16)
    inp("iota", [P, P])
    inp("w_es1", [ED + 1 + ND, ND], F16)
    inp("w_e2", [ED + 1, ND], F16)
    inp("w11", [ND, ND]); inp("b11", [ND, 1])
    inp("w12", [ND, ND]); inp("b12", [ND, 1])
    inp("w21", [ND, EMB]); inp("b21", [EMB, 1])
    inp("w22", [EMB, EMB]); inp("b22", [EMB, 1])
    inp("relg", [P, NT])
    inp("pool_idx", [P, cfg.n_pool_idx // 16], I16)
    inp("cnt_gm", [P, GSP // P]); inp("usrT", [USR, GSP])
    for nm, shp in (("hw1a", [EMB, 128]), ("hw1b", [USR, 128]), ("hb1", [128, 1]),
                    ("hw2", [128, 64]), ("hb2", [64, 1]), ("hw3", [64, 32]),
                    ("hb3", [32, 1]), ("hw4", [32, 16]), ("hb4", [16, 1]),
                    ("hw5", [16, 1]), ("hb5", [1, 1])):
        inp(nm, shp)
    yT = nc.dram_tensor("yT", [1, GSP], F32, kind="ExternalOutput").ap()

    NROW = NT * GW + P
    kseq, offs = cfg.kseq, cfg.offs

    with tile.TileContext(nc) as tc, ExitStack() as ctx:
        const = ctx.enter_context(tc.tile_pool(name="const", bufs=1))

        def ld(name, shape, dt=F32):
            t = const.tile(shape, dt, name=f"c_{name}")
            nc.sync.dma_start(t[:], d[name])
            return t

        w_es1 = ld("w_es1", [ED + 1 + ND, ND], F16)
        w_e2 = ld("w_e2", [ED + 1, ND], F16)
        w11 = ld("w11", [ND, ND]); b11 = ld("b11", [ND, 1])
        w12 = ld("w12", [ND, ND]); b12 = ld("b12", [ND, 1])
        w21 = ld("w21", [ND, EMB]); b21 = ld("b21", [EMB, 1])
        w22 = ld("w22", [EMB, EMB]); b22 = ld("b22", [EMB, 1])
        iota = ld("iota", [P, P])
        relg = ld("relg", [P, NT])
        ident = const.tile([P, P], F32, name="ident")
        make_identity(nc, ident[:])
        identh = const.tile([P, P], F16, name="identh")
        nc.vector.tensor_copy(out=identh[:], in_=ident[:])
        zt = const.tile([P, 256], F32, name="zt")
        nc.vector.memset(zt[:], 0.0)

        dram = ctx.enter_context(tc.tile_pool(name="dram", bufs=1, space="DRAM"))
        h1T_d = dram.tile([ND, N_SH], F32)
        a2a_in = dram.tile([C * ND, 2 * B_h], F16)
        a2a_out = dram.tile([C * ND, 2 * B_h], F16)
        parts = dram.tile([NROW, P], F32)
        nc.sync.dma_start(
            out=parts[NT * GW:NT * GW + P, :].rearrange("(p r) e -> p (r e)", p=P),
            in_=zt[:, :P])

        tabp = ctx.enter_context(tc.tile_pool(name="tabp", bufs=1))

        def edge_chunk(pools, mode, tab_t, w_e, residT_d, chunk, out_cb,
                       relu_last, w1, b1, w2, b2, HID1):
            """Process one chunk of node tiles: messages, aggregate,
            node MLP; out_cb(col0, nb, out_tile) with out [HID, nb] at
            node cols [col0, col0+nb)."""
            (gp, xp, ep, dp, ip, mp, msp, hp, z1p, o1p,
             psl, pst, psa, psm) = pools
            lo, ntiles = chunk
            e0, e1 = int(offs[lo]), int(offs[lo + ntiles])
            CH = e1 - e0
            KB = CH // P
            dsr = dp.tile([P, KB], F32, tag="dsr")
            nc.sync.dma_start(dsr[:], d["dstrel"][:, e0 // P:e1 // P])
            xr = hp.tile([ND, ntiles * P], F32, tag="xr")
            nc.sync.dma_start(xr[:], residT_d[:, lo * P:(lo + ntiles) * P])
            ind = ip.tile([P, KB * P], F16, tag="ind")
            nc.vector.tensor_tensor(
                out=ind[:].rearrange("p (j n) -> p j n", n=P),
                in0=dsr[:].unsqueeze(2).broadcast_to([P, KB, P]),
                in1=iota[:].unsqueeze(1).broadcast_to([P, KB, P]),
                op=ALU.is_equal)
            msge = msp.tile([P, KB * ND], F16, tag="msge")
            if mode == "stream":
                es = ep.tile([ED + 1 + ND, CH], F16, tag="es")
                nc.sync.dma_start(es[:], d["estream"][:, e0:e1])
                # fused edge-linear + x_src add (identity rows); 2 tiles/bank
                for j0 in range(0, KB, 2):
                    nj = min(2, KB - j0)
                    ps = psl.tile([P, 2 * ND], F32, tag="lin")
                    for j in range(nj):
                        nc.tensor.matmul(
                            out=ps[:, j * ND:(j + 1) * ND],
                            lhsT=es[:, (j0 + j) * P:(j0 + j + 1) * P],
                            rhs=w_e[:], start=True, stop=True)
                    nc.scalar.activation(
                        out=msge[:, j0 * ND:(j0 + nj) * ND],
                        in_=ps[:, :nj * ND], func=AF.Relu)
            else:
                gix = gp.tile([P, CH // 16], I16, tag="gix")
                nc.sync.dma_start(gix[:], d["gidx2"][:, e0 // 16:e1 // 16])
                xg = xp.tile([P, CH], F16, tag="xg")
                nc.gpsimd.dma_gather(
                    out_ap=xg[:].rearrange("p (o c) -> p o c", o=1),
                    in_ap=tab_t[:], idxs_ap=gix[:],
                    num_idxs=CH, num_idxs_reg=CH, elem_size=P,
                    transpose=True, single_packet=False,
                    sbuf_tokens_per_rank=P,
                    sbuf_free_dim_per_rank=256)
                eat = ep.tile([ED + 1, CH], F16, tag="es")
                nc.sync.dma_start(eat[:], d["eaT"][:, e0:e1])
                # edge-major: ea@W accumulated with x_src via identity rhs
                for j0 in range(0, KB, 2):
                    nj = min(2, KB - j0)
                    ps = psl.tile([P, 2 * ND], F32, tag="lin")
                    for j in range(nj):
                        jj = j0 + j
                        nc.tensor.matmul(out=ps[:, j * ND:(j + 1) * ND],
                                         lhsT=eat[:, jj * P:(jj + 1) * P],
                                         rhs=w_e[:], start=True, stop=False)
                        nc.tensor.matmul(out=ps[:, j * ND:(j + 1) * ND],
                                         lhsT=xg[:ND, jj * P:(jj + 1) * P],
                                         rhs=identh[:ND, :ND],
                                         start=False, stop=True)
                    nc.scalar.activation(out=msge[:, j0 * ND:(j0 + nj) * ND],
                                         in_=ps[:, :nj * ND], func=AF.Relu)
            # aggregate + node residual per tile
            hT = hp.tile([ND, ntiles * P], F32, tag="hT")
            for t in range(ntiles):
                i = lo + t
                j0 = int(offs[i] - e0) // P
                k = int(kseq[i])
                pa = psa.tile([ND, P], F32, tag="agg")
                for j in range(k):
                    nc.tensor.matmul(out=pa[:],
                                     lhsT=msge[:, (j0 + j) * ND:(j0 + j + 1) * ND],
                                     rhs=ind[:, (j0 + j) * P:(j0 + j + 1) * P],
                                     start=(j == 0), stop=(j == k - 1))
                nc.vector.tensor_add(out=hT[:, t * P:(t + 1) * P], in0=pa[:],
                                     in1=xr[:, t * P:(t + 1) * P])
            # node MLP in 512-col blocks
            for b0 in range(0, ntiles * P, 512):
                nb = min(512, ntiles * P - b0)
                z1ps = psm.tile([HID1, 512], F32, tag="mlp1")
                nc.tensor.matmul(out=z1ps[:, :nb], lhsT=w1[:],
                                 rhs=hT[:, b0:b0 + nb], start=True, stop=True)
                z1 = z1p.tile([HID1, 512], F32, tag="z1")
                nc.scalar.activation(out=z1[:, :nb], in_=z1ps[:, :nb],
                                     func=AF.Relu, bias=b1[:])
                if w2 is not None:
                    z2ps = psm.tile([HID1, 512], F32, tag="mlp2")
                    nc.tensor.matmul(out=z2ps[:, :nb], lhsT=w2[:],
                                     rhs=z1[:, :nb], start=True, stop=True)
                    o1 = o1p.tile([HID1, 512], F32, tag="o1")
                    nc.scalar.activation(
                        out=o1[:, :nb], in_=z2ps[:, :nb],
                        func=AF.Relu if relu_last else AF.Identity, bias=b2[:])
                    out_cb(lo * P + b0, nb, o1)
                else:
                    out_cb(lo * P + b0, nb, z1)

        # ================= layer 1 =================
        with ExitStack() as e1:
            h1p = e1.enter_context(tc.tile_pool(name="h1p", bufs=1))
            segb = cfg.segb
            h1seg = [h1p.tile([P, (segb[q + 1] - segb[q]) * P], F16,
                              name=f"h1seg{q}") for q in range(cfg.SEG)]

            gp = e1.enter_context(tc.tile_pool(name="gp", bufs=2))
            xp = e1.enter_context(tc.tile_pool(name="xp", bufs=2))
            ep = e1.enter_context(tc.tile_pool(name="ep", bufs=2))
            dp = e1.enter_context(tc.tile_pool(name="dp", bufs=2))
            ip = e1.enter_context(tc.tile_pool(name="ip", bufs=2))
            mp = e1.enter_context(tc.tile_pool(name="mp", bufs=2))
            msp = e1.enter_context(tc.tile_pool(name="msp", bufs=2))
            hp = e1.enter_context(tc.tile_pool(name="hp", bufs=2))
            z1p = e1.enter_context(tc.tile_pool(name="z1p", bufs=2))
            o1p = e1.enter_context(tc.tile_pool(name="o1p", bufs=2))
            psl = e1.enter_context(tc.tile_pool(name="psl", bufs=2, space="PSUM"))
            pst = e1.enter_context(tc.tile_pool(name="pst", bufs=2, space="PSUM"))
            psa = e1.enter_context(tc.tile_pool(name="psa", bufs=2, space="PSUM"))
            psm = e1.enter_context(tc.tile_pool(name="psm", bufs=1, space="PSUM"))
            pools = (gp, xp, ep, dp, ip, mp, msp, hp, z1p, o1p,
                     psl, pst, psa, psm)

            import bisect

            def l1_out(col0, nb, o1):
                # h1 block [64, nb] at node cols [col0, col0+nb)
                nc.sync.dma_start(
                    out=h1T_d[:, col0:col0 + nb], in_=o1[:ND, :nb])
                for tt in range(nb // P):
                    pos = col0 // P + tt
                    q = bisect.bisect_right(segb, pos) - 1
                    sl = pos - segb[q]
                    tp = pst.tile([P, P], F32, tag="tp")
                    nc.tensor.transpose(out=tp[:, :ND],
                                        in_=o1[:ND, tt * P:(tt + 1) * P],
                                        identity=ident[:ND, :ND])
                    nc.scalar.activation(
                        out=h1seg[q][:, sl * P:sl * P + ND],
                        in_=tp[:, :ND], func=AF.Identity)

            for chunk in cfg.chunksA + cfg.chunksB:
                edge_chunk(pools, "stream", None, w_es1, d["xT"], chunk,
                           l1_out, True, w11, b11, w12, b12, ND)

            # ---- a2a sends ----
            sgp = e1.enter_context(tc.tile_pool(name="sgp", bufs=2))
            agp = e1.enter_context(tc.tile_pool(name="agp", bufs=2))
            offQ, BQ = cfg.offQ, cfg.BQ
            for q in range(cfg.SEG):
                o0 = int(offQ[q])
                for dest in range(C):
                    for h in range(2):
                        s0 = (dest * 2 + h) * B_h
                        six = sgp.tile([P, BQ[q] // 16], I16, tag="six")
                        nc.sync.dma_start(
                            six[:],
                            d["sg"][:, (s0 + o0) // 16:(s0 + o0 + BQ[q]) // 16])
                        gt = agp.tile([P, max(BQ)], F16, tag="gt")
                        nc.gpsimd.dma_gather(
                            out_ap=gt[:, :BQ[q]]
                            .rearrange("p (o c) -> p o c", o=1),
                            in_ap=h1seg[q][:], idxs_ap=six[:],
                            num_idxs=BQ[q], num_idxs_reg=BQ[q], elem_size=P,
                            transpose=True, single_packet=False,
                            sbuf_tokens_per_rank=P,
                            sbuf_free_dim_per_rank=256)
                        nc.sync.dma_start(
                            out=a2a_in[dest * ND:(dest + 1) * ND,
                                       h * B_h + o0:h * B_h + o0 + BQ[q]],
                            in_=gt[:ND, :BQ[q]])
            nc.gpsimd.collective_compute(
                "AllToAll", mybir.AluOpType.bypass,
                replica_groups=[list(range(C))],
                ins=[a2a_in[:].opt()], outs=[a2a_out[:].opt()])

        # ================= layer 2 =================
        with ExitStack() as e2:
            gp = e2.enter_context(tc.tile_pool(name="gp2", bufs=2))
            xp = e2.enter_context(tc.tile_pool(name="xp2", bufs=2))
            ep = e2.enter_context(tc.tile_pool(name="ep2", bufs=2))
            dp = e2.enter_context(tc.tile_pool(name="dp2", bufs=2))
            ip = e2.enter_context(tc.tile_pool(name="ip2", bufs=2))
            mp = e2.enter_context(tc.tile_pool(name="mp2", bufs=2))
            msp = e2.enter_context(tc.tile_pool(name="msp2", bufs=2))
            hp = e2.enter_context(tc.tile_pool(name="hp2", bufs=2))
            z1p = e2.enter_context(tc.tile_pool(name="z1p2", bufs=2))
            o1p = e2.enter_context(tc.tile_pool(name="o1p2", bufs=2))
            stp = e2.enter_context(tc.tile_pool(name="stp", bufs=2))
            z2rp = e2.enter_context(tc.tile_pool(name="z2rp", bufs=2))
            sip = e2.enter_context(tc.tile_pool(name="sip", bufs=2))
            pbp = e2.enter_context(tc.tile_pool(name="pbp", bufs=2))
            psl = e2.enter_context(tc.tile_pool(name="psl2", bufs=2, space="PSUM"))
            pst = e2.enter_context(tc.tile_pool(name="pst2", bufs=2, space="PSUM"))
            psa = e2.enter_context(tc.tile_pool(name="psa2", bufs=2, space="PSUM"))
            psm = e2.enter_context(tc.tile_pool(name="psm2", bufs=1, space="PSUM"))
            pools = (gp, xp, ep, dp, ip, mp, msp, hp, z1p, o1p,
                     psl, pst, psa, psm)

            def build_tab(half):
                t = tabp.tile([P, C * B_h], F16, tag="xtab")
                for o in range(C):
                    st = stp.tile([ND, B_h], F16, tag="stage")
                    nc.sync.dma_start(
                        st[:], a2a_out[o * ND:(o + 1) * ND,
                                       half * B_h:(half + 1) * B_h])
                    for s in range(B_h // P):
                        tp = pst.tile([P, P], F16, tag="tp")
                        nc.tensor.transpose(out=tp[:, :ND],
                                            in_=st[:, s * P:(s + 1) * P],
                                            identity=identh[:ND, :ND])
                        pos = o * B_h // P + s
                        nc.scalar.activation(
                            out=t[:, pos * P:pos * P + ND],
                            in_=tp[:, :ND], func=AF.Identity)
                return t

            def l2_out(col0, nb, z1):
                # z1 block [128, nb] at node cols [col0, col0+nb); pool per tile
                for tt in range(nb // P):
                    i = col0 // P + tt
                    z2ps = pst.tile([P, P], F32, tag="tp")
                    nc.tensor.matmul(out=z2ps[:], lhsT=z1[:, tt * P:(tt + 1) * P],
                                     rhs=w22[:], start=True, stop=True)
                    z2r = z2rp.tile([P, P], F32, tag="z2r")
                    nc.scalar.activation(out=z2r[:], in_=z2ps[:],
                                         func=AF.Identity)
                    S = sip.tile([P, GW], F32, tag="S")
                    nc.vector.tensor_tensor(
                        out=S[:], in0=relg[:, i:i + 1].to_broadcast([P, GW]),
                        in1=iota[:, :GW], op=ALU.is_equal)
                    pp = psm.tile([GW, P], F32, tag="part")
                    nc.tensor.matmul(out=pp[:], lhsT=S[:], rhs=z2r[:],
                                     start=True, stop=True)
                    psb = pbp.tile([GW, P], F32, tag="psb")
                    nc.vector.tensor_copy(out=psb[:], in_=pp[:])
                    nc.sync.dma_start(out=parts[i * GW:(i + 1) * GW, :],
                                      in_=psb[:])

            tab2A = build_tab(0)
            for chunk in cfg.chunksA:
                edge_chunk(pools, "gather", tab2A, w_e2, h1T_d[:], chunk,
                           l2_out, False, w21, b21, None, None, EMB)
            tab2B = build_tab(1)
            for chunk in cfg.chunksB:
                edge_chunk(pools, "gather", tab2B, w_e2, h1T_d[:], chunk,
                           l2_out, False, w21, b21, None, None, EMB)

        # ================= pool reduce + head =================
        with ExitStack() as hctx:
            hp = hctx.enter_context(tc.tile_pool(name="hd", bufs=1))
            hps = hctx.enter_context(tc.tile_pool(name="hd_ps", bufs=2, space="PSUM"))
            pix = hp.tile([P, cfg.n_pool_idx // 16], I16)
            nc.sync.dma_start(pix[:], d["pool_idx"])
            NPB = cfg.n_pool_idx // P
            gpo = hp.tile([P, NPB * P], F32)
            nc.gpsimd.dma_gather(
                out_ap=gpo[:].rearrange("p (k e) -> p k e", e=P),
                in_ap=parts[:], idxs_ap=pix[:],
                num_idxs=cfg.n_pool_idx, num_idxs_reg=cfg.n_pool_idx,
                elem_size=P, single_packet=False)
            GB = GSP // P
            v = gpo[:].rearrange("p (q b e) -> p q b e", q=PG, b=GB)
            pooled = hp.tile([P, GB * P], F32)
            pv = pooled[:].rearrange("p (b e) -> p b e", b=GB)
            if PG == 1:
                nc.vector.tensor_copy(out=pv, in_=v[:, 0])
            else:
                nc.vector.tensor_add(out=pv, in0=v[:, 0], in1=v[:, 1])
                for q in range(2, PG):
                    nc.vector.tensor_add(out=pv, in0=pv, in1=v[:, q])
            cntg = hp.tile([P, GB], F32)
            nc.sync.dma_start(cntg[:], d["cnt_gm"])
            invc = hp.tile([P, GB], F32)
            nc.vector.reciprocal(invc[:], cntg[:])
            for b in range(GB):
                nc.vector.tensor_tensor(
                    out=pooled[:, b * P:(b + 1) * P],
                    in0=pooled[:, b * P:(b + 1) * P],
                    in1=invc[:, b:b + 1].to_broadcast([P, P]), op=ALU.mult)
            embT = hp.tile([P, GSP], F32)
            for b in range(GB):
                tps = hps.tile([P, P], F32, tag="hd")
                nc.tensor.transpose(out=tps[:], in_=pooled[:, b * P:(b + 1) * P],
                                    identity=ident[:])
                nc.scalar.activation(out=embT[:, b * P:(b + 1) * P], in_=tps[:],
                                     func=AF.Identity, bias=b22[:])
            usrT = hp.tile([USR, GSP], F32)
            nc.sync.dma_start(usrT[:], d["usrT"])
            hw = {nm: hp.tile(d[nm].shape, F32, name=f"t_{nm}")
                  for nm in ("hw1a", "hw1b", "hb1", "hw2", "hb2", "hw3", "hb3",
                             "hw4", "hb4", "hw5", "hb5")}
            for nm, t in hw.items():
                nc.sync.dma_start(t[:], d[nm])
            z1ps = hps.tile([128, GSP], F32, tag="hd")
            nc.tensor.matmul(out=z1ps[:], lhsT=hw["hw1a"][:], rhs=embT[:],
                             start=True, stop=False)
            nc.tensor.matmul(out=z1ps[:], lhsT=hw["hw1b"][:], rhs=usrT[:],
                             start=False, stop=True)
            z1 = hp.tile([128, GSP], F32)
            nc.scalar.activation(out=z1[:], in_=z1ps[:], func=AF.Relu, bias=hw["hb1"][:])
            z2ps = hps.tile([64, GSP], F32, tag="hd")
            nc.tensor.matmul(out=z2ps[:], lhsT=hw["hw2"][:], rhs=z1[:], start=True, stop=True)
            z2 = hp.tile([64, GSP], F32)
            nc.scalar.activation(out=z2[:], in_=z2ps[:], func=AF.Relu, bias=hw["hb2"][:])
            z3ps = hps.tile([32, GSP], F32, tag="hd")
            nc.tensor.matmul(out=z3ps[:], lhsT=hw["hw3"][:], rhs=z2[:], start=True, stop=True)
            z3 = hp.tile([32, GSP], F32)
            nc.scalar.activation(out=z3[:], in_=z3ps[:], func=AF.Relu, bias=hw["hb3"][:])
            z4ps = hps.tile([16, GSP], F32, tag="hd")
            nc.tensor.matmul(out=z4ps[:], lhsT=hw["hw4"][:], rhs=z3[:], start=True, stop=True)
            z4 = hp.tile([16, GSP], F32)
            nc.scalar.activation(out=z4[:], in_=z4ps[:], func=AF.Relu, bias=hw["hb4"][:])
            z5ps = hps.tile([1, GSP], F32, tag="hd")
            nc.tensor.matmul(out=z5ps[:], lhsT=hw["hw5"][:], rhs=z4[:], start=True, stop=True)
            z5 = hp.tile([1, GSP], F32)
            nc.scalar.activation(out=z5[:], in_=z5ps[:], func=AF.Identity, bias=hw["hb5"][:])
            nc.sync.dma_start(out=yT, in_=z5[:])

    nc.compile()
    return nc


def _make_in_maps(cfg, cores, inputs):
    f32 = lambda a: np.ascontiguousarray(np.asarray(a, np.float32))
    f16 = lambda a: np.ascontiguousarray(np.asarray(a, np.float16))
    usr = f32(inputs["usr"])
    w_es1 = np.vstack([f32(inputs["e1_w"]), f32(inputs["e1_b"])[None, :],
                       np.eye(ND, dtype=np.float32)])
    w_e2 = np.vstack([f32(inputs["e2_w"]), f32(inputs["e2_b"])[None, :]])
    iota = np.tile(np.arange(P, dtype=np.float32), (P, 1))
    in_maps = []
    for c, pc in enumerate(cores):
        usrT = np.zeros((USR, GSP), np.float32)
        usrT[:, :GS] = usr[c * GS:(c + 1) * GS].T
        in_maps.append(dict(
            estream=pc["estream"],
            gidx2=_wrap16(pc["gidx2"]),
            eaT=f16(pc["eaT"]),
            dstrel=np.ascontiguousarray(
                pc["dstrel"].reshape(cfg.ESH // P, P).T),
            xT=pc["xT"], sg=_wrap16(pc["sg"]), iota=iota,
            w_es1=f16(w_es1), w_e2=f16(w_e2),
            w11=f32(inputs["n1_w1"]), b11=f32(inputs["n1_b1"])[:, None],
            w12=f32(inputs["n1_w2"]), b12=f32(inputs["n1_b2"])[:, None],
            w21=f32(inputs["n2_w1"]), b21=f32(inputs["n2_b1"])[:, None],
            w22=f32(inputs["n2_w2"]), b22=f32(inputs["n2_b2"])[:, None],
            relg=np.ascontiguousarray(pc["relg"].T),
            pool_idx=_wrap16(pc["pool_idx"]),
            cnt_gm=pc["cnt_gm"], usrT=usrT,
            hw1a=f32(inputs["h1_w"])[:EMB], hw1b=f32(inputs["h1_w"])[EMB:],
            hb1=f32(inputs["h1_b"])[:, None],
            hw2=f32(inputs["h2_w"]), hb2=f32(inputs["h2_b"])[:, None],
            hw3=f32(inputs["h3_w"]), hb3=f32(inputs["h3_b"])[:, None],
            hw4=f32(inputs["h4_w"]), hb4=f32(inputs["h4_b"])[:, None],
            hw5=f32(inputs["h5_w"]), hb5=f32(inputs["h5_b"])[:, None]))
    return in_maps


def kernel(**inputs):
    cfg, gb, cores = _preprocess(
        inputs["x"], inputs["edge_index"], inputs["edge_attr"], inputs["batch"])
    nc = _build(cfg)
    in_maps = _make_in_maps(cfg, cores, inputs)
    res = bass_utils.run_bass_kernel_spmd(nc, in_maps, core_ids=list(range(C)))
    out = np.concatenate([res.results[c]["yT"][0, :GS] for c in range(C)])
    kernel._last = res
    return out[:, None].astype(np.float32)
